# revision 65
# baseline (speedup 1.0000x reference)
"""Trainium2 Bass kernel for nn_PoolWithHole: 3x3 max-pool excluding the
center tap, zero-padded borders, clamped at 0:

    out[b,i,j] = max(0, max_{(di,dj)!=(0,0), |di|<=1, |dj|<=1} x[b,i+di,j+dj])

Sharding: data parallel over batch B=64 -> 8 NeuronCores x 8 images.
v5: 130.5us/core vs v4's 146.8us (1.13x) vs the exact fp32 v1's 380.8us
(2.9x); rel err ~1.1e-3 from fp16 rounding (harness gate is 2e-2).

v4 was DVE-bound (134us busy, ACT 130) with a 1/3-newpath / 2/3-oldpath
block mix.  v5 rebalances all five engines (DVE/ACT/PE ~117-119us busy,
DMA ~110us):

  * All-newpath supertiles: ACT's evacuation cost is the same 2
    passes/block on BOTH the old and the relu-max path, so converting
    old-path blocks to newpath cuts one DVE pass/block for free (at the
    price of PE, which had headroom).
  * D-block (block 5 of each supertile): needs NO PE/ACT at all.  The
    vertical shifts come from partition-shifted SBUF->SBUF DMA copies
    (DMA had ~25us headroom): S2c[p] = h[p+2], S1c[p] = m2c[p+1], with
    m2c = max(m2, 0) via a tensor_scalar clamp (DVE 4x perf mode, 0.30
    ns/elem/lane).  Combine is two plain fp16 2x-mode tensor_tensor
    maxes: Wt = max(h, S2c); out = max(Wt, S1c) (S1c >= 0 supplies the
    zero clamp, so no extra relu anywhere).
  * Single-W PSUM chain tiles: the newpath used to hold D|E in one
    [126, 2W] fp32 PSUM tile (4 banks), limiting PSUM to 2 block-chains
    in flight; the PE->ACT->PE->ACT chain coupling then cost ~30us of
    bubbles (and PE p-state down-ramps: the tensor engine runs at 1/2
    clock for ~3us after every idle gap).  v5 computes D = S2h - S1m in
    a [126, W] tile (2 banks) and lets E = S1m + relu(D) OVERWRITE the
    same banks after r is evacuated -- 4 chains in flight, PE/ACT stay
    gapless (worth 17us).  rcp holds 4 r-buffers to match.
  * m2/h row ops split into two 3-block halves: with subtile deps the
    chain for blocks 0-2 starts while the DVE still computes blocks 3-5
    (worth 3.3us of PE continuity).
  * Weights DMA is issued first (PE's first Ldweights gated the fill).

Other v4 machinery kept: fp16 compute end-to-end (host converts), images
stacked into one tall column with shared zero separator rows (the zero
clamp makes the extra zero neighbors harmless), 126-output-row blocks on
128 partitions, 6-block supertiles (6144-wide DVE ops), lag-2 software
pipeline (combines for tile t issue at iteration t+2), tile 0 split
per-block for fast pipeline fill, tail tiles' combines split per-block
so out-DMAs overlap the drain, block 65 transfers only its 9 useful
rows.
"""

import os
import sys

sys.path.insert(0, "/opt/trn_rl_repo")
os.environ.setdefault("MYCRO_LOCAL_CACHE", "1")

import numpy as np
from contextlib import ExitStack

import concourse.bass as bass  # noqa: F401  (registers AP machinery)
from concourse import bacc, mybir
import concourse.tile as tile
from concourse import bass_utils

F16 = mybir.dt.float16
F32 = mybir.dt.float32
MAX = mybir.AluOpType.max
RELU = mybir.ActivationFunctionType.Relu

_APC = None


def _ap_class():
    global _APC
    if _APC is None:
        _APC = type(
            bass.Bass("TRN2", target_bir_lowering=False)
            .alloc_sbuf_tensor("_apq", [1, 1], F16)
            .ap()
        )
    return _APC


def _mkap(base, doffset, dims):
    """Arbitrary affine AP into base's tensor: dims = [[step, count], ...]."""
    return _ap_class()(base.tensor, base.offset + doffset, dims)


N_CORES = 8
FULL_B, H, W = 64, 1024, 1024
B_LOCAL = FULL_B // N_CORES

TO = 126            # output rows per block (128-row input slab)
G = 6               # blocks per supertile (free-dim concatenation)
NBLK = 66           # ceil(8192 / 126) -> 66, and 66 == 11 * G exactly
NTILES = NBLK // G
NN = 5              # newpath blocks per tile (block 5 is the D-block)
XBW = G * (W + 1) + 1      # X tile width: G data blocks + shared zero cols
R_IN = NBLK * TO + 2       # stacked input rows (8318)
R_OUT = NBLK * TO          # stacked output rows (8316)
STRIDE_S = W + 1           # image row i r -> stacked row 1 + i*(1024+1) + r

_NC_CACHE: dict = {}


def shift_matrices() -> np.ndarray:
    """lhsT bank [128, 504]: cols 0:126 shift-by-2 (S2h), 126:252
    shift-by-1 (S1m), 252:378 negated shift-by-1 (for D = S2h - S1m via
    PSUM accumulation), 378:504 identity (to accumulate relu(D) back
    onto S1m: S1m + relu(S2h - S1m) = max(S2h, S1m)).

    out = lhsT.T @ rhs, so lhsT[k, p] = w puts w*rhs[k] into out[p]."""
    m = np.zeros((128, 4 * TO), dtype=np.float16)
    for p in range(TO):
        m[p + 2, p] = 1.0
        m[p + 1, TO + p] = 1.0
        m[p + 1, 2 * TO + p] = -1.0
        m[p, 3 * TO + p] = 1.0
    return m


def build_nc():
    nc = bacc.Bacc(
        "TRN2",
        target_bir_lowering=False,
        debug=False,
        enable_asserts=False,
        num_devices=N_CORES,
    )
    x = nc.dram_tensor("x", [R_IN, W], F16, kind="ExternalInput").ap()
    shm = nc.dram_tensor("shm", [128, 4 * TO], F16, kind="ExternalInput").ap()
    out = nc.dram_tensor("out", [R_OUT, W], F16, kind="ExternalOutput").ap()

    with tile.TileContext(nc) as tc, ExitStack() as ctx:
        cp = ctx.enter_context(tc.tile_pool(name="const", bufs=1))
        xp = ctx.enter_context(tc.tile_pool(name="xp", bufs=1))
        m2p = ctx.enter_context(tc.tile_pool(name="m2p", bufs=2))
        hp = ctx.enter_context(tc.tile_pool(name="hp", bufs=3))
        m2cp = ctx.enter_context(tc.tile_pool(name="m2cp", bufs=2))
        dcp = ctx.enter_context(tc.tile_pool(name="dcp", bufs=3))
        dop = ctx.enter_context(tc.tile_pool(name="dop", bufs=3))
        vcp = ctx.enter_context(tc.tile_pool(name="vcp", bufs=3))
        rcp = ctx.enter_context(tc.tile_pool(name="rcp", bufs=4))
        wtp = ctx.enter_context(tc.tile_pool(name="wtp", bufs=3))
        x0p = ctx.enter_context(tc.tile_pool(name="x0p", bufs=1))
        pp = ctx.enter_context(tc.tile_pool(name="psum", bufs=4, space="PSUM"))

        SH = cp.tile([128, 4 * TO], F16)

        # Persistent X buffers: zero separator columns at multiples of
        # W+1 memset once; per-tile DMAs write only the data columns.
        xbufs = []
        for i in range(3):
            Xi = xp.tile([128, XBW], F16, tag=f"Xb{i}")
            nc.gpsimd.memset(Xi[:, 0 : XBW : W + 1], 0.0)
            xbufs.append(Xi)
        # Tile 0 uses separate one-block X tiles so each per-block m2/h
        # sub-op waits only on its own block's DMA (fast pipeline fill).
        # Block 0 itself is two independent half-tiles (3-col halo overlap)
        # so the very first DVE op waits on a single ~128KB DMA.
        XH1 = x0p.tile([128, 518], F16, tag="X0h1")  # [z | x cols 0..516]
        nc.gpsimd.memset(XH1[:, 0:1], 0.0)
        XH2 = x0p.tile([128, 512], F16, tag="X0h2")  # [x cols 513..1023 | z]
        nc.gpsimd.memset(XH2[:, 511:512], 0.0)
        X0b1 = x0p.tile([128, W + 2], F16, tag="X0b1")
        nc.gpsimd.memset(X0b1[:, 0 : W + 2 : W + 1], 0.0)
        # Tile 0 also gets per-block M2/Hh tiles: the matmul->ACT chain for
        # block g then depends only on that block's h (deps are
        # tile-granular), so the first ACT chain starts right after the
        # first block's h instead of after the whole tile.
        m2t0 = []
        ht0 = []
        for g in range(4):
            Mg = x0p.tile([128, W], F16, tag=f"M2t0{g}")
            m2t0.append(Mg)
            Hg = x0p.tile([128, W], F16, tag=f"Ht0{g}")
            ht0.append(Hg)

        # Two-stage software pipeline: combines for tile t are issued at
        # iteration t+2, interleaved between m2/h of tile t+2 so that
        # consecutive DVE instructions are independent (the in-order DVE
        # never stalls on the ScalarE evacuation chain or on its own
        # ack-return latency).
        hist: dict = {}
        for t in range(NTILES + 2):
            if t < NTILES:
                if t == 0:
                    X = xbufs[0]

                    def _xdma(g):
                        nc.sync.dma_start(
                            X[:, g * (W + 1) + 1 : g * (W + 1) + 1 + W],
                            x[g * TO : g * TO + 128, :],
                        )

                    # wire order: block-0 first half (gates the first
                    # DVE op), weights (PE's first Ldweights), rest
                    nc.sync.dma_start(XH1[:, 1:518], x[0:128, 0:517])
                    nc.sync.dma_start(XH2[:, 0:511], x[0:128, 513:1024])
                    nc.sync.dma_start(SH[:, :], shm[:, :])
                    _xdma(2)
                    _xdma(3)
                    nc.sync.dma_start(X0b1[:, 1 : 1 + W], x[TO : TO + 128, :])
                    _xdma(4)
                    _xdma(5)
                else:
                    X = xbufs[t % 3]
                    for g in range(G):
                        b = t * G + g
                        rows = 11 if b == NBLK - 1 else 128
                        nc.sync.dma_start(
                            X[0:rows, g * (W + 1) + 1 : g * (W + 1) + 1 + W],
                            x[b * TO : b * TO + rows, :],
                        )

                # m2[j] = max(x[j-1], x[j+1])  (hole-row max), all G blocks
                # in one op via a block-strided 3D AP; innermost dim
                # unit-stride so the DVE 2x mode applies.  Tile 0 is split
                # into per-block sub-ops on separate X tiles so the DVE
                # starts as soon as the first block's DMA lands.
                M2 = m2p.tile([128, G * W], F16)
                m2b = M2[:, :]
                if t == 0:
                    xparts = [(0, 0, 514, XH1[:, :], 0, 1), (0, 514, 510, XH2[:, :], 0, 1)]
                    xparts += [(g, 0, W, X[:, :], g * (W + 1), 1) for g in range(2, 4)]
                    xparts += [(1, 0, W, X0b1[:, :], 0, 1)]
                    xparts += [(None, g * W, W, X[:, :], g * (W + 1), 1) for g in range(4, G)]
                else:
                    # two 3-block halves: with subtile deps the PE chain
                    # for blocks 0-2 starts while the DVE still computes
                    # the second half (keeps PE out of idle/p-state dips)
                    xparts = [(None, 0, W, X[:, :], 0, 3),
                              (None, 3 * W, W, X[:, :], 3 * (W + 1), 3)]

                def _m2_op(part):
                    gi, co, cn, xb, xo, gn = part
                    dst = m2t0[gi][:, :] if gi is not None else m2b
                    pstep = xb.ap[0][0]
                    nc.vector.tensor_tensor(
                        _mkap(dst, co, [[dst.ap[0][0], 128], [W, gn], [1, cn]]),
                        _mkap(xb, xo, [[pstep, 128], [W + 1, gn], [1, cn]]),
                        _mkap(xb, xo + 2, [[pstep, 128], [W + 1, gn], [1, cn]]),
                        MAX,
                    )

                def _h_op(part, hb):
                    gi, co, cn, xb, xo, gn = part
                    dst = ht0[gi][:, :] if gi is not None else hb
                    m2src = m2t0[gi][:, :] if gi is not None else m2b
                    nc.vector.tensor_tensor(
                        _mkap(dst, co, [[dst.ap[0][0], 128], [W, gn], [1, cn]]),
                        _mkap(m2src, co, [[m2src.ap[0][0], 128], [W, gn], [1, cn]]),
                        _mkap(xb, xo + 1, [[xb.ap[0][0], 128], [W + 1, gn], [1, cn]]),
                        MAX,
                    )

                if t == 0:
                    # interleave m2/h per block with lag 1 (no adjacent DVE
                    # deps) so the first ACT evac chain starts early
                    Hh = hp.tile([128, G * W], F16)
                    hb = Hh[:, :]
                    _m2_op(xparts[0])
                    for i in range(1, len(xparts)):
                        _m2_op(xparts[i])
                        _h_op(xparts[i - 1], hb)
                    _h_op(xparts[-1], hb)
                else:
                    for part in xparts:
                        _m2_op(part)

                # D-block clamp: m2c = max(m2[block 5], 0) --
                # tensor_scalar runs in the DVE 4x perf mode.  Emitted
                # between m2 and h so the DVE reaches h (which gates the
                # whole PE chain for this tile) as early as possible --
                # keeping PE gapless keeps it out of the slow p-states.
                M2C = m2cp.tile([128, W], F16)
                nc.vector.tensor_scalar(
                    M2C[:, :], M2[:, NN * W : G * W], 0.0, None, MAX
                )

            if 0 < t < NTILES:
                # h[j] = max(m2[j], x[j])  (full 3-tap row max, reusing m2)
                Hh = hp.tile([128, G * W], F16)
                hb = Hh[:, :]
                for part in xparts:
                    _h_op(part, hb)

            if t >= 2:
                # Combines for tile t-2 (all after h of tile t): newpath
                # finals Wt = max(h, V) and the D-block's two DMA-shift
                # maxes, ordered so no two adjacent DVE ops are dependent.
                hb_p, vcb_p, s2c_p, s1c_p = hist[t - 2]
                # only the LAST tile needs per-block drain splitting; tile
                # 9's out-DMAs still have tile-10 compute to hide behind
                tail = t >= NTILES + 1
                Wt = wtp.tile([126, NN * W], F16)
                wtb = Wt[:, :]

                def _final(g0, gn):
                    if t == 2 and g0 < 4:
                        for g in range(g0, g0 + gn):
                            hsrc = ht0[g][:, :] if g < 4 else hb_p
                            hoff = 0 if g < 4 else g * W
                            nc.vector.tensor_tensor(
                                _mkap(wtb, g * W, [[wtb.ap[0][0], 126], [1, W]]),
                                _mkap(hsrc, hoff, [[hsrc.ap[0][0], 126], [1, W]]),
                                _mkap(vcb_p, g * W, [[vcb_p.ap[0][0], 126], [1, W]]),
                                MAX,
                            )
                    else:
                        nc.vector.tensor_tensor(
                            _mkap(wtb, g0 * W, [[wtb.ap[0][0], 126], [1, gn * W]]),
                            _mkap(hb_p, g0 * W, [[hb_p.ap[0][0], 126], [1, gn * W]]),
                            _mkap(vcb_p, g0 * W, [[vcb_p.ap[0][0], 126], [1, gn * W]]),
                            MAX,
                        )

                def _odma(g0, gn):
                    if tail:
                        for g in range(g0, g0 + gn):
                            b = (t - 2) * G + g
                            nc.sync.dma_start(
                                out[b * TO : (b + 1) * TO, :],
                                Wt[:, g * W : (g + 1) * W],
                            )
                    else:
                        b0 = (t - 2) * G + g0
                        nc.sync.dma_start(
                            _mkap(out[:, :], b0 * TO * W,
                                  [[W, 126], [TO * W, gn], [1, W]]),
                            _mkap(wtb, g0 * W,
                                  [[wtb.ap[0][0], 126], [W, gn], [1, W]]),
                        )

                _final(0, 3)
                _odma(0, 3)
                WD = dop.tile([126, W], F16)
                nc.vector.tensor_tensor(
                    _mkap(WD[:, :], 0, [[WD[:, :].ap[0][0], 126], [1, W]]),
                    _mkap(hb_p, NN * W, [[hb_p.ap[0][0], 126], [1, W]]),
                    s2c_p,
                    MAX,
                )
                _final(3, 2)
                _odma(3, 2)
                nc.vector.tensor_tensor(WD[:, :], WD[:, :], s1c_p, MAX)
                b = (t - 2) * G + NN
                rows = 9 if b == NBLK - 1 else TO
                nc.sync.dma_start(
                    out[b * TO : b * TO + rows, :], WD[0:rows, :]
                )

            if t < NTILES:
                # D-block shift copies (SBUF->SBUF partition-shifted DMA):
                # S2c[p] = h[p+2, block5], S1c[p] = m2c[p+1].  Issued after
                # h so they execute during tile t+1, consumed at t+2.
                hsrc5 = Hh
                S2C = dcp.tile([126, W], F16, tag="S2C")
                S1C = dcp.tile([126, W], F16, tag="S1C")
                nc.sync.dma_start(S2C[:, :], hsrc5[2:128, NN * W : G * W])
                nc.sync.dma_start(S1C[:, :], M2C[1:127, :])

                # Newpath blocks 0..4: PE computes D = S2h - S1m (shift +
                # negated-shift accumulated in PSUM) next to E = S1m; ACT
                # evacuates r = relu(D); PE accumulates r onto E (identity
                # matmul) giving E = max(S2h, S1m); ACT evacuates
                # V = relu(E) = max(h[p+2], m2[p+1], 0).  The DVE then needs
                # only ONE combine pass: out = max(h[p], V).  Emission is
                # lag-1 interleaved so ACT never waits on the PE accumulate
                # round-trip.
                VC = vcp.tile([126, NN * W], F16, tag="VC")

                def _src(g):
                    if t == 0 and g < 4:
                        return ht0[g], m2t0[g], 0
                    return Hh, M2, g * W

                qs = {}
                rs = {}

                def _de_mms(g):
                    # D = S2h - S1m in a single-W PSUM tile (2 banks);
                    # E reuses the SAME banks after r is evacuated, so 4
                    # block-chains fit in PSUM concurrently (vs 2 with the
                    # [126, 2W] D|E layout) -- keeps PE/ACT gapless.
                    hsrc, msrc, off = _src(g)
                    Q = pp.tile([126, W], F32, tag="P")
                    qs[g] = Q
                    for c in range(0, W, 512):
                        nc.tensor.matmul(
                            Q[:, c : c + 512], SH[:, 0:TO],
                            hsrc[:, off + c : off + c + 512],
                            start=True, stop=False,
                        )
                    for c in range(0, W, 512):
                        nc.tensor.matmul(
                            Q[:, c : c + 512], SH[:, 2 * TO : 3 * TO],
                            msrc[:, off + c : off + c + 512],
                            start=False, stop=True,
                        )

                def _r_act(g):
                    R = rcp.tile([126, W], F16)
                    rs[g] = R
                    nc.scalar.activation(R[:, :], qs[g][:, :], RELU)

                def _acc_mms(g):
                    # E = S1m + identity @ r  (overwrites D's banks)
                    hsrc, msrc, off = _src(g)
                    for c in range(0, W, 512):
                        nc.tensor.matmul(
                            qs[g][:, c : c + 512], SH[:, TO : 2 * TO],
                            msrc[:, off + c : off + c + 512],
                            start=True, stop=False,
                        )
                    for c in range(0, W, 512):
                        nc.tensor.matmul(
                            qs[g][:, c : c + 512],
                            SH[0:126, 3 * TO : 4 * TO],
                            rs[g][:, c : c + 512],
                            start=False, stop=True,
                        )

                def _v_act(g):
                    nc.scalar.activation(
                        VC[:, g * W : (g + 1) * W], qs[g][:, :], RELU
                    )

                # lag-1 interleaved chain over the 5 newpath blocks
                _de_mms(0)
                _r_act(0)
                for g in range(1, NN):
                    _de_mms(g)
                    _acc_mms(g - 1)
                    _r_act(g)
                    if g >= 1:
                        _v_act(g - 1)
                _acc_mms(NN - 1)
                _v_act(NN - 1)

                hist[t] = (hb, VC[:, :], S2C[:, :], S1C[:, :])

    nc.compile()
    return nc


def _get_nc():
    if "nc" not in _NC_CACHE:
        _NC_CACHE["nc"] = build_nc()
    return _NC_CACHE["nc"]


def _stack_core(xc: np.ndarray) -> np.ndarray:
    """[B_LOCAL, H, W] fp32 -> stacked [R_IN, W] fp16 with zero separator
    rows; image i row r lands at stacked row 1 + i*(H+1) + r."""
    xs = np.zeros((R_IN, W), dtype=np.float16)
    body = xs[1 : 1 + B_LOCAL * (H + 1)].reshape(B_LOCAL, H + 1, W)
    body[:, :H, :] = xc.astype(np.float16)
    return xs


def kernel(x: np.ndarray, **_unused) -> np.ndarray:
    """Full-input entry point: x [64,1024,1024] fp32 -> out same shape."""
    x = np.asarray(x)
    assert x.shape == (FULL_B, H, W), x.shape
    nc = _get_nc()
    shm = shift_matrices()
    in_maps = [
        {"x": _stack_core(x[i * B_LOCAL : (i + 1) * B_LOCAL]), "shm": shm}
        for i in range(N_CORES)
    ]
    res = bass_utils.run_bass_kernel_spmd(
        nc, in_maps, core_ids=list(range(N_CORES))
    )
    outs = []
    for r in res.results:
        os_ = np.asarray(r["out"])  # [R_OUT, W] fp16
        per = os_[: B_LOCAL * (H + 1)].reshape(B_LOCAL, H + 1, W)[:, :H, :]
        outs.append(per.astype(np.float32))
    return np.concatenate(outs, axis=0)
